# revision 28
# baseline (speedup 1.0000x reference)
"""minGRU cell on 8 Trainium2 NeuronCores.

Math (per batch sample, per hidden channel):
    gh    = x @ W.T + b              # (S, 2H): [gate | hidden]
    z_t   = sigmoid(gate_t)
    a_t   = 1 - z_t = sigmoid(-gate_t)
    g_t   = hidden_t + 0.5  if hidden_t >= 0 else sigmoid(hidden_t)
          = min(sigmoid(hidden_t), 0.5) + relu(hidden_t)
    h_t   = a_t * h_{t-1} + z_t * g_t        # linear first-order recurrence

Sharding: data-parallel over batch B=8, one sample per core.

Device layout is channel-major ([channel, time]) everywhere so that
 - the matmul contraction dim (IN) sits on partitions for both operands
   (host pre-transposes x and W — free on host, avoids on-chip transposes),
 - the recurrence runs along the free dim, which is exactly what the DVE
   tensor_tensor_scan instruction implements (fp32 state feedback).
The host transposes the channel-major fp16 result back to (B, S, H) fp32.

Pipeline per (512-wide s-tile, 128-channel block cb): PE matmul -> PSUM;
ACT: a = sigmoid(-(gate+b_g)) [fp16], zh = sigmoid(hid+b_h), r = relu(hid+b_h)
(r alternates between ACT and DVE for engine balance); DVE at 2048-wide group
grain: z = 1-a, c = min(zh, .5), g = c+r, b = z*g, scan h = a*h + b chained
through a carry column; one fp16 store per (group, cb) = 8 stores on 8 SWDGE
lanes.

The walrus codegen for this toolchain rejects instructions whose sync-wait
list exceeds a (small, per-ISA-struct) budget — effectively ONE wait for
Activation / Matmult / scan / TensorCopy / DMA pseudo-instructions. The
guard instructions below absorb cross-engine waits into each engine's
in-order observed clock so every real instruction needs at most one wait.
"""

import numpy as np

try:
    import concourse.bass as bass
except ImportError:  # pragma: no cover
    import sys

    sys.path.insert(0, "/opt/trn_rl_repo")
    import concourse.bass as bass

import concourse.mybir as mybir
from concourse.bass_utils import run_bass_kernel_spmd
from concourse.tile import TileContext, add_dep_helper

B, S, IN, H = 8, 8192, 256, 256
N_CORES = 8
SW = 512  # s-tile width (one PSUM bank)
NS = S // SW
G = 4  # s-tiles per group (DVE grain + one store per group => 8 stores)
GW = G * SW
NG = NS // G

_F16 = mybir.dt.float16
_F32 = mybir.dt.float32


def _build():
    nc = bass.Bass()
    Op = mybir.AluOpType
    AF = mybir.ActivationFunctionType

    xT = nc.declare_dram_parameter("xT", [IN, S], _F16, isOutput=False)
    Wt = nc.declare_dram_parameter("Wt", [IN, 2 * H], _F16, isOutput=False)
    bias = nc.declare_dram_parameter("bias", [H, 2], _F32, isOutput=False)
    h0 = nc.declare_dram_parameter("h0", [H, 1], _F32, isOutput=False)
    hT = nc.declare_dram_parameter("hT", [H, S], _F16, isOutput=True)

    with TileContext(nc) as tc:
        with (
            tc.tile_pool(name="const", bufs=1) as cpool,
            tc.tile_pool(name="xin", bufs=NS) as xpool,
            tc.tile_pool(name="work", bufs=2) as wpool,
            tc.tile_pool(name="psum", bufs=2, space="PSUM") as ppool,
        ):
            w_sb = []
            for k in range(2):
                wt = cpool.tile([128, 2 * H], _F16, name=f"w{k}")
                nc.sync.dma_start(out=wt, in_=Wt[k * 128 : (k + 1) * 128, :])
                w_sb.append(wt)
            bias_sb = []
            h0_sb = []
            for cb in range(2):
                bt = cpool.tile([128, 2], _F32, name=f"bias{cb}")
                nc.sync.dma_start(out=bt, in_=bias[cb * 128 : (cb + 1) * 128, :])
                bias_sb.append(bt)
                ht = cpool.tile([128, 1], _F32, name=f"h0{cb}")
                nc.sync.dma_start(out=ht, in_=h0[cb * 128 : (cb + 1) * 128, :])
                h0_sb.append(ht)

            # h0 through a DVE copy: first scans' carry dep becomes same-engine.
            carry = []
            for cb in range(2):
                c0 = cpool.tile([128, 1], _F32, name=f"carry{cb}")
                nc.vector.tensor_copy(out=c0, in_=h0_sb[cb])
                carry.append(c0[:, 0:1])

            # ACT observes the bias DMAs once up front.
            warm0 = cpool.tile([128, 2], _F32, name="warm0")
            warm1 = cpool.tile([128, 2], _F32, name="warm1")
            nc.scalar.copy(out=warm0, in_=bias_sb[0])
            nc.scalar.copy(out=warm1, in_=bias_sb[1])
            # DVE observes the bias DMAs too (its relu ops read the bias AP).
            fd0 = cpool.tile([128, 2], _F16, name="fd0")
            fd1 = cpool.tile([128, 2], _F16, name="fd1")
            nc.vector.tensor_copy(out=fd0, in_=bias_sb[0])
            nc.vector.tensor_copy(out=fd1, in_=bias_sb[1])

            a_w = [None, None]
            zh_w = [None, None]
            r_w = [None, None]
            hg_hist = [[], []]
            act_hist = []  # per-si ACT psum readers
            dver_hist = []  # per-si DVE psum readers
            store_hist = []  # per-group [store_cb0, store_cb1]
            all_loads = []
            last_scans = []
            last_mms = []

            for si in range(NS):
                u, j = divmod(si, G)
                act_hist.append([])
                dver_hist.append([])

                # PE guards absorb the psum-bank releases (readers of the
                # banks two s-tiles ago: ACT ops, and DVE relu ops).
                pe_guards = []
                if si >= 2:
                    pg = nc.tensor.ldweights(w_sb[0][:, 0:1])
                    for ai in act_hist[si - 2]:
                        add_dep_helper(pg.ins, ai.ins, True, "psum release act")
                    pe_guards.append(pg)
                    if dver_hist[si - 2]:
                        pg2 = nc.tensor.ldweights(w_sb[1][:, 0:1])
                        for di in dver_hist[si - 2]:
                            add_dep_helper(pg2.ins, di.ins, True, "psum release dve")
                        pe_guards.append(pg2)

                xk = []
                for k in range(2):
                    xt = xpool.tile([128, SW], _F16, name="xt", tag=f"x{k}")
                    ld = nc.sync.dma_start(
                        out=xt, in_=xT[k * 128 : (k + 1) * 128, si * SW : (si + 1) * SW]
                    )
                    all_loads.append(ld)
                    xk.append(xt)

                # DVE guards: observe the store whose group tile gets recycled
                # by this group's scan (fired two groups back).
                dve_guards = []
                if j == 0 and u >= 2:
                    for cb in range(2):
                        dscr = cpool.tile([128, 1], _F16, name=f"dscr_{u}_{cb}")
                        gdv = nc.vector.tensor_copy(out=dscr, in_=fd0[:, 0:1])
                        add_dep_helper(
                            gdv.ins, store_hist[u - 2][cb].ins, True, "observe store"
                        )
                        dve_guards.append(gdv)

                # ACT guards: ratchet ACT's observed DVE clock past the scan
                # of the wide tiles being recycled (bufs=2 -> group u-2).
                guards = []
                if j == 0 and u >= 2:
                    for cb in range(2):
                        scr = cpool.tile([128, 1], _F16, name=f"scr_{u}_{cb}")
                        gd = nc.scalar.copy(out=scr, in_=hg_hist[cb][u - 2][:, 0:1])
                        guards.append(gd)

                for cb in range(2):
                    if j == 0:
                        a_w[cb] = wpool.tile([128, GW], _F16, name="a_w", tag=f"a{cb}")
                        zh_w[cb] = wpool.tile(
                            [128, GW], _F16, name="zh_w", tag=f"zh{cb}"
                        )
                        r_w[cb] = wpool.tile([128, GW], _F16, name="r_w", tag=f"r{cb}")
                    sl = slice(j * SW, (j + 1) * SW)
                    g_ps = ppool.tile([128, SW], _F32, name="g_ps", tag=f"gp{cb}")
                    h_ps = ppool.tile([128, SW], _F32, name="h_ps", tag=f"hp{cb}")
                    for k in range(2):
                        mm = nc.tensor.matmul(
                            g_ps,
                            w_sb[k][:, cb * 128 : (cb + 1) * 128],
                            xk[k],
                            start=(k == 0),
                            stop=(k == 1),
                        )
                        for pg in pe_guards:
                            add_dep_helper(mm.ins, pg.ins, False, "after guard")
                        if si == NS - 1:
                            last_mms.append(mm)
                    for k in range(2):
                        mm = nc.tensor.matmul(
                            h_ps,
                            w_sb[k][:, H + cb * 128 : H + (cb + 1) * 128],
                            xk[k],
                            start=(k == 0),
                            stop=(k == 1),
                        )
                        for pg in pe_guards:
                            add_dep_helper(mm.ins, pg.ins, False, "after guard")
                        if si == NS - 1:
                            last_mms.append(mm)
                    # a = sigmoid(-(gate + b_g))  [fp16 scan coefficient]
                    a_inst = nc.scalar.activation(
                        a_w[cb][:, sl],
                        g_ps,
                        AF.Sigmoid,
                        bias=bias_sb[cb][:, 0:1],
                        scale=-1.0,
                    )
                    for gd in guards:
                        add_dep_helper(a_inst.ins, gd.ins, False, "guard before ACT")
                    act_hist[si].append(a_inst)
                    # zh = sigmoid(hidden + b_h)
                    act_hist[si].append(
                        nc.scalar.activation(
                            zh_w[cb][:, sl],
                            h_ps,
                            AF.Sigmoid,
                            bias=bias_sb[cb][:, 1:2],
                            scale=1.0,
                        )
                    )
                    # r = relu(hidden + b_h): alternate engines for balance
                    if j % 2 == 0:
                        act_hist[si].append(
                            nc.scalar.activation(
                                r_w[cb][:, sl],
                                h_ps,
                                AF.Relu,
                                bias=bias_sb[cb][:, 1:2],
                                scale=1.0,
                            )
                        )
                    else:
                        dver_hist[si].append(
                            nc.vector.tensor_scalar(
                                r_w[cb][:, sl],
                                h_ps,
                                bias_sb[cb][:, 1:2],
                                0.0,
                                Op.add,
                                Op.max,
                            )
                        )

                    if j == G - 1:
                        # group-grain DVE chain over [128, GW]
                        z_w = wpool.tile([128, GW], _F16, name="z_w", tag=f"z{cb}")
                        nc.vector.tensor_scalar(
                            z_w, a_w[cb], -1.0, 1.0, Op.mult, Op.add
                        )
                        c_w = wpool.tile([128, GW], _F16, name="c_w", tag=f"c{cb}")
                        nc.vector.tensor_scalar_min(c_w, zh_w[cb], 0.5)
                        gg_w = wpool.tile([128, GW], _F16, name="gg_w", tag=f"gg{cb}")
                        nc.vector.tensor_add(gg_w, c_w, r_w[cb])
                        b_w = wpool.tile([128, GW], _F16, name="b_w", tag=f"b{cb}")
                        nc.vector.tensor_mul(b_w, z_w, gg_w)
                        hg = wpool.tile([128, GW], _F16, name="hg", tag=f"hg{cb}")
                        sc_inst = nc.vector.tensor_tensor_scan(
                            hg, a_w[cb], b_w, carry[cb], Op.mult, Op.add
                        )
                        for gdv in dve_guards:
                            add_dep_helper(sc_inst.ins, gdv.ins, True, "join guards")
                        if si == NS - 1:
                            last_scans.append(sc_inst)
                        hg_hist[cb].append(hg)
                        carry[cb] = hg[:, GW - 1 : GW]
                        st = nc.gpsimd.dma_start(
                            out=hT[cb * 128 : (cb + 1) * 128, u * GW : (u + 1) * GW],
                            in_=hg,
                        )
                        if cb == 0:
                            store_hist.append([st])
                        else:
                            store_hist[u].append(st)

            # Pre-absorb the kernel-tail drain's waits (single-dep nofuse SP
            # nops: the control-instruction wait budget is tiny).
            tail_deps = (
                [s for pair in store_hist for s in pair]
                + all_loads[-8:]
                + last_scans
                + last_mms
                + act_hist[NS - 1]
                + dver_hist[NS - 1]
            )
            for d in tail_deps:
                tn = nc.sync.nop(nofuse=True)
                add_dep_helper(tn.ins, d.ins, True, "tail absorb")
    return nc


_NC_CACHE = None


def _get_nc():
    global _NC_CACHE
    if _NC_CACHE is None:
        _NC_CACHE = _build()
    return _NC_CACHE


def _prepare_in_maps(x, h0, W, b):
    x = np.asarray(x, dtype=np.float32)
    h0 = np.asarray(h0, dtype=np.float32)
    W = np.asarray(W, dtype=np.float32)
    b = np.asarray(b, dtype=np.float32)

    Wt = np.ascontiguousarray(W.T).astype(np.float16)  # [IN, 2H]
    bias_pack = np.ascontiguousarray(
        np.stack([-b[:H], b[H:]], axis=1).astype(np.float32)
    )  # [H, 2]: col0 = -b_gate, col1 = b_hidden

    in_maps = []
    for i in range(N_CORES):
        xTi = x[i].T.astype(np.float16, order="C")  # [IN, S]
        h0i = np.ascontiguousarray(h0[i, 0].reshape(H, 1))  # [H, 1]
        in_maps.append({"xT": xTi, "Wt": Wt, "bias": bias_pack, "h0": h0i})
    return in_maps


def _run(x, h0, W, b, trace=False):
    nc = _get_nc()
    in_maps = _prepare_in_maps(x, h0, W, b)
    res = run_bass_kernel_spmd(nc, in_maps, list(range(N_CORES)), trace=trace)
    out = np.empty((B, S, H), dtype=np.float32)
    for i in range(N_CORES):
        out[i] = res.results[i]["hT"].T.astype(np.float32)
    h_next = out[:, -1:, :].copy()
    return (out, h_next), res


def kernel(x, h0, W, b):
    (out, h_next), _ = _run(x, h0, W, b, trace=False)
    return out, h_next


# revision 31
# speedup vs baseline: 1.0861x; 1.0861x over previous
"""minGRU cell on 8 Trainium2 NeuronCores.

Math (per batch sample, per hidden channel):
    gh    = x @ W.T + b              # (S, 2H): [gate | hidden]
    z_t   = sigmoid(gate_t)
    a_t   = 1 - z_t = sigmoid(-gate_t)
    g_t   = hidden_t + 0.5  if hidden_t >= 0 else sigmoid(hidden_t)
          = min(sigmoid(hidden_t), 0.5) + relu(hidden_t)
    h_t   = a_t * h_{t-1} + z_t * g_t        # linear first-order recurrence

Sharding: data-parallel over batch B=8, one sample per core.

Device layout is channel-major ([channel, time]) everywhere so that
 - the matmul contraction dim (IN) sits on partitions for both operands
   (host pre-transposes x and W — free on host, avoids on-chip transposes),
 - the recurrence runs along the free dim, which is exactly what the DVE
   tensor_tensor_scan instruction implements (fp32 state feedback).
The host transposes the channel-major fp16 result back to (B, S, H) fp32.

Pipeline per (512-wide s-tile, 128-channel block cb): PE matmul -> PSUM;
ACT: a = sigmoid(-(gate+b_g)) [fp16], zh = sigmoid(hid+b_h), r = relu(hid+b_h)
(r alternates between ACT and DVE for engine balance); DVE at 2048-wide group
grain: z = 1-a, c = min(zh, .5), g = c+r, b = z*g, scan h = a*h + b chained
through a carry column; one fp16 store per (group, cb) = 8 stores on 8 SWDGE
lanes.

The walrus codegen for this toolchain rejects instructions whose sync-wait
list exceeds a (small, per-ISA-struct) budget — effectively ONE wait for
Activation / Matmult / scan / TensorCopy / DMA pseudo-instructions. The
guard instructions below absorb cross-engine waits into each engine's
in-order observed clock so every real instruction needs at most one wait.
"""

import numpy as np

try:
    import concourse.bass as bass
except ImportError:  # pragma: no cover
    import sys

    sys.path.insert(0, "/opt/trn_rl_repo")
    import concourse.bass as bass

import concourse.mybir as mybir
from concourse.bass_utils import run_bass_kernel_spmd
from concourse.tile import TileContext, add_dep_helper

B, S, IN, H = 8, 8192, 256, 256
N_CORES = 8
SW = 512  # s-tile width (one PSUM bank)
NS = S // SW
G = 4  # s-tiles per store group (one store per group => 8 stores)
GW = G * SW
HW2 = 2 * SW  # DVE chain sub-grain
NG = NS // G

_F16 = mybir.dt.float16
_F32 = mybir.dt.float32


def _build():
    nc = bass.Bass()
    Op = mybir.AluOpType
    AF = mybir.ActivationFunctionType

    xT = nc.declare_dram_parameter("xT", [IN, S], _F16, isOutput=False)
    Wt = nc.declare_dram_parameter("Wt", [IN, 2 * H], _F16, isOutput=False)
    bias = nc.declare_dram_parameter("bias", [H, 2], _F32, isOutput=False)
    h0 = nc.declare_dram_parameter("h0", [H, 1], _F32, isOutput=False)
    hT = nc.declare_dram_parameter("hT", [H, S], _F16, isOutput=True)

    with TileContext(nc) as tc:
        with (
            tc.tile_pool(name="const", bufs=1) as cpool,
            tc.tile_pool(name="xin", bufs=NS) as xpool,
            tc.tile_pool(name="work", bufs=2) as wpool,
            tc.tile_pool(name="psum", bufs=2, space="PSUM") as ppool,
        ):
            w_sb = []
            for k in range(2):
                wt = cpool.tile([128, 2 * H], _F16, name=f"w{k}")
                nc.sync.dma_start(out=wt, in_=Wt[k * 128 : (k + 1) * 128, :])
                w_sb.append(wt)
            bias_sb = []
            h0_sb = []
            for cb in range(2):
                bt = cpool.tile([128, 2], _F32, name=f"bias{cb}")
                nc.sync.dma_start(out=bt, in_=bias[cb * 128 : (cb + 1) * 128, :])
                bias_sb.append(bt)
                ht = cpool.tile([128, 1], _F32, name=f"h0{cb}")
                nc.sync.dma_start(out=ht, in_=h0[cb * 128 : (cb + 1) * 128, :])
                h0_sb.append(ht)

            # h0 through a DVE copy: first scans' carry dep becomes same-engine.
            carry = []
            for cb in range(2):
                c0 = cpool.tile([128, 1], _F32, name=f"carry{cb}")
                nc.vector.tensor_copy(out=c0, in_=h0_sb[cb])
                carry.append(c0[:, 0:1])

            # ACT observes the bias DMAs once up front.
            warm0 = cpool.tile([128, 2], _F32, name="warm0")
            warm1 = cpool.tile([128, 2], _F32, name="warm1")
            nc.scalar.copy(out=warm0, in_=bias_sb[0])
            nc.scalar.copy(out=warm1, in_=bias_sb[1])
            # DVE observes the bias DMAs too (its relu ops read the bias AP).
            fd0 = cpool.tile([128, 2], _F16, name="fd0")
            fd1 = cpool.tile([128, 2], _F16, name="fd1")
            nc.vector.tensor_copy(out=fd0, in_=bias_sb[0])
            nc.vector.tensor_copy(out=fd1, in_=bias_sb[1])

            a_w = [None, None]
            hg_cur = [None, None]
            zh_w = [None, None]
            r_w = [None, None]
            hg_hist = [[], []]
            act_hist = []  # per-si ACT psum readers
            dver_hist = []  # per-si DVE psum readers
            store_hist = []  # per-group [store_cb0, store_cb1]
            all_loads = []
            last_scans = []
            last_mms = []

            for si in range(NS):
                u, j = divmod(si, G)
                act_hist.append([])
                dver_hist.append([])

                # PE guards absorb the psum-bank releases (readers of the
                # banks two s-tiles ago: ACT ops, and DVE relu ops).
                pe_guards = []
                if si >= 2:
                    pg = nc.tensor.ldweights(w_sb[0][:, 0:1])
                    for ai in act_hist[si - 2]:
                        add_dep_helper(pg.ins, ai.ins, True, "psum release act")
                    pe_guards.append(pg)

                xk = []
                for k in range(2):
                    xt = xpool.tile([128, SW], _F16, name="xt", tag=f"x{k}")
                    ld = nc.sync.dma_start(
                        out=xt, in_=xT[k * 128 : (k + 1) * 128, si * SW : (si + 1) * SW]
                    )
                    all_loads.append(ld)
                    xk.append(xt)

                # DVE guards: observe the store whose group tile gets recycled
                # by this group's scan (fired two groups back).
                dve_guards = []
                if j == 0 and u >= 2:
                    for cb in range(2):
                        dscr = cpool.tile([128, 1], _F16, name=f"dscr_{u}_{cb}")
                        gdv = nc.vector.tensor_copy(out=dscr, in_=fd0[:, 0:1])
                        add_dep_helper(
                            gdv.ins, store_hist[u - 2][cb].ins, True, "observe store"
                        )
                        dve_guards.append(gdv)

                # ACT guards: ratchet ACT's observed DVE clock past the scan
                # of the wide tiles being recycled (bufs=2 -> group u-2).
                guards = []
                if j == 0 and u >= 2:
                    for cb in range(2):
                        scr = cpool.tile([128, 1], _F16, name=f"scr_{u}_{cb}")
                        gd = nc.scalar.copy(
                            out=scr, in_=hg_hist[cb][u - 2][:, GW - 1 : GW]
                        )
                        guards.append(gd)

                for cb in range(2):
                    if j == 0:
                        a_w[cb] = wpool.tile([128, GW], _F16, name="a_w", tag=f"a{cb}")
                        zh_w[cb] = wpool.tile(
                            [128, GW], _F16, name="zh_w", tag=f"zh{cb}"
                        )
                        r_w[cb] = wpool.tile([128, GW], _F16, name="r_w", tag=f"r{cb}")
                    sl = slice(j * SW, (j + 1) * SW)
                    g_ps = ppool.tile([128, SW], _F32, name="g_ps", tag=f"gp{cb}")
                    h_ps = ppool.tile([128, SW], _F32, name="h_ps", tag=f"hp{cb}")
                    for k in range(2):
                        mm = nc.tensor.matmul(
                            g_ps,
                            w_sb[k][:, cb * 128 : (cb + 1) * 128],
                            xk[k],
                            start=(k == 0),
                            stop=(k == 1),
                        )
                        for pg in pe_guards:
                            add_dep_helper(mm.ins, pg.ins, False, "after guard")
                        if si == NS - 1:
                            last_mms.append(mm)
                    for k in range(2):
                        mm = nc.tensor.matmul(
                            h_ps,
                            w_sb[k][:, H + cb * 128 : H + (cb + 1) * 128],
                            xk[k],
                            start=(k == 0),
                            stop=(k == 1),
                        )
                        for pg in pe_guards:
                            add_dep_helper(mm.ins, pg.ins, False, "after guard")
                        if si == NS - 1:
                            last_mms.append(mm)
                    # a = sigmoid(-(gate + b_g))  [fp16 scan coefficient]
                    a_inst = nc.scalar.activation(
                        a_w[cb][:, sl],
                        g_ps,
                        AF.Sigmoid,
                        bias=bias_sb[cb][:, 0:1],
                        scale=-1.0,
                    )
                    for gd in guards:
                        add_dep_helper(a_inst.ins, gd.ins, False, "guard before ACT")
                    act_hist[si].append(a_inst)
                    # r = relu(hidden + b_h)
                    r_inst = nc.scalar.activation(
                        r_w[cb][:, sl],
                        h_ps,
                        AF.Relu,
                        bias=bias_sb[cb][:, 1:2],
                        scale=1.0,
                    )
                    act_hist[si].append(r_inst)
                    # zh = sigmoid(hidden + b_h) — forced LAST in ACT order:
                    # the DVE chain's first op waits on zh and thereby
                    # observes a and r too (one wait per instruction).
                    zh_inst = nc.scalar.activation(
                        zh_w[cb][:, sl],
                        h_ps,
                        AF.Sigmoid,
                        bias=bias_sb[cb][:, 1:2],
                        scale=1.0,
                    )
                    add_dep_helper(zh_inst.ins, r_inst.ins, False, "zh last")
                    add_dep_helper(zh_inst.ins, a_inst.ins, False, "zh last")
                    act_hist[si].append(zh_inst)

                    if j == 1:
                        hg = wpool.tile([128, GW], _F16, name="hg", tag=f"hg{cb}")
                        hg_cur[cb] = hg
                        hg_hist[cb].append(hg)
                    if j % 2 == 1:
                        # half-group DVE chain over [128, HW2]
                        hsl = slice((j - 1) * SW, (j + 1) * SW)
                        hg = hg_cur[cb]
                        c_w = wpool.tile([128, HW2], _F16, name="c_w", tag=f"c{cb}")
                        c_inst = nc.vector.tensor_scalar_min(c_w, zh_w[cb][:, hsl], 0.5)
                        z_w = wpool.tile([128, HW2], _F16, name="z_w", tag=f"z{cb}")
                        zi = nc.vector.tensor_scalar(
                            z_w, a_w[cb][:, hsl], -1.0, 1.0, Op.mult, Op.add
                        )
                        add_dep_helper(zi.ins, c_inst.ins, False, "c first")
                        gg_w = wpool.tile([128, HW2], _F16, name="gg_w", tag=f"gg{cb}")
                        nc.vector.tensor_add(gg_w, c_w, r_w[cb][:, hsl])
                        b_w = wpool.tile([128, HW2], _F16, name="b_w", tag=f"b{cb}")
                        nc.vector.tensor_mul(b_w, z_w, gg_w)
                        sc_inst = nc.vector.tensor_tensor_scan(
                            hg[:, hsl], a_w[cb][:, hsl], b_w, carry[cb], Op.mult, Op.add
                        )
                        for gdv in dve_guards:
                            add_dep_helper(sc_inst.ins, gdv.ins, True, "join guards")
                        if si == NS - 1:
                            last_scans.append(sc_inst)
                        carry[cb] = hg[:, (j + 1) * SW - 1 : (j + 1) * SW]
                    if j == G - 1:
                        hg = hg_cur[cb]
                        st = nc.gpsimd.dma_start(
                            out=hT[cb * 128 : (cb + 1) * 128, u * GW : (u + 1) * GW],
                            in_=hg,
                        )
                        if cb == 0:
                            store_hist.append([st])
                        else:
                            store_hist[u].append(st)

            # Pre-absorb the kernel-tail drain's waits (single-dep nofuse SP
            # nops: the control-instruction wait budget is tiny).
            tail_deps = (
                [s for pair in store_hist for s in pair]
                + all_loads[-8:]
                + last_scans
                + last_mms
                + act_hist[NS - 1]
            )
            for d in tail_deps:
                tn = nc.sync.nop(nofuse=True)
                add_dep_helper(tn.ins, d.ins, True, "tail absorb")
    return nc


_NC_CACHE = None


def _get_nc():
    global _NC_CACHE
    if _NC_CACHE is None:
        _NC_CACHE = _build()
    return _NC_CACHE


def _prepare_in_maps(x, h0, W, b):
    x = np.asarray(x, dtype=np.float32)
    h0 = np.asarray(h0, dtype=np.float32)
    W = np.asarray(W, dtype=np.float32)
    b = np.asarray(b, dtype=np.float32)

    Wt = np.ascontiguousarray(W.T).astype(np.float16)  # [IN, 2H]
    bias_pack = np.ascontiguousarray(
        np.stack([-b[:H], b[H:]], axis=1).astype(np.float32)
    )  # [H, 2]: col0 = -b_gate, col1 = b_hidden

    in_maps = []
    for i in range(N_CORES):
        xTi = x[i].T.astype(np.float16, order="C")  # [IN, S]
        h0i = np.ascontiguousarray(h0[i, 0].reshape(H, 1))  # [H, 1]
        in_maps.append({"xT": xTi, "Wt": Wt, "bias": bias_pack, "h0": h0i})
    return in_maps


def _run(x, h0, W, b, trace=False):
    nc = _get_nc()
    in_maps = _prepare_in_maps(x, h0, W, b)
    res = run_bass_kernel_spmd(nc, in_maps, list(range(N_CORES)), trace=trace)
    out = np.empty((B, S, H), dtype=np.float32)
    for i in range(N_CORES):
        out[i] = res.results[i]["hT"].T.astype(np.float32)
    h_next = out[:, -1:, :].copy()
    return (out, h_next), res


def kernel(x, h0, W, b):
    (out, h_next), _ = _run(x, h0, W, b, trace=False)
    return out, h_next
